# revision 6
# baseline (speedup 1.0000x reference)
"""Trainium2 Bass kernel for nn_CustomPokemonModel.

Model (per sample): embedding gathers (12 ability + 48 move slots) + small
linear on numericals -> concat x [B, 7808] -> single-step LSTM (4*128 gates)
-> 2-layer MLP head -> 9-way masked softmax.  Returns (out, h1, c1).

Strategy: pure data parallel over 8 NeuronCores (512 samples each).  On each
core the whole pipeline is computed feature-major ("xT" layout: features on
partitions, samples on the free dim) so every matmul contracts over the
partition dim with B=512 on the moving free dim:

  - embeddings are gathered row-wise with dma_gather (sample-major blocks of
    [128 samples, 128 feat]) and transposed on the PE with an identity matmul
  - gates^T [4x128, 512] accumulate in 4 PSUM banks over 61 k-chunks of
    Wih^T (streamed from DRAM) plus the Whh @ h0^T term
  - LSTM elementwise + MLP head stay feature-major; the final 9-way logits
    flip back to sample-major by using feat2^T 128-sample chunks as the
    stationary operand; softmax + mask renorm run on DVE/ACT
  - h1/c1 are PE-transposed back to sample-major for output

MM_DTYPE selects the PE dtype for the big matmuls: float32 (exact, 4 cyc/row)
or float32r (full-speed fp32-storage mode, 1 cyc/row at N>=512).
"""
import sys

sys.path.insert(0, "/opt/trn_rl_repo")

import numpy as np

import concourse.bass as bass
import concourse.tile as tile
from concourse import bacc, bass_utils, mybir
from concourse.masks import make_identity

f32 = mybir.dt.float32
i16 = mybir.dt.int16
AF = mybir.ActivationFunctionType

N_CORES = 8
B = 4096
BL = B // N_CORES          # 512 samples per core
ED = 128
LH = 128
HD = 64
NG = 4                      # gate chunks (i, f, g, o)
N_AB, N_MV = 12, 48
K_CHUNKS = 61               # 12 ability + 48 move + 1 numerical
N_GRP = 15                  # gather/matmul groups of 4 slots
SLOT_G = 4                  # slots per group
IDX_PER_G = SLOT_G * BL     # 2048 indices per dma_gather
BCH = BL // 128             # 4 sample chunks of 128

# "f32" = exact fp32 matmuls; "f32r" = float32r (fast fp32-storage) matmuls
MM_DTYPE = "f32"


def _mm_ap(ap):
    if MM_DTYPE == "f32r":
        return ap.bitcast(mybir.dt.float32r)
    return ap


_CACHE = {}


def _build():
    key = MM_DTYPE
    if key in _CACHE:
        return _CACHE[key]

    nc = bacc.Bacc("TRN2", target_bir_lowering=False, debug=False)

    dt = nc.dram_tensor
    wihT = dt("wihT", [K_CHUNKS, 128, NG * LH], f32, kind="ExternalInput").ap()
    ab_emb = dt("ab_emb", [300, ED], f32, kind="ExternalInput").ap()
    mv_emb = dt("mv_emb", [900, ED], f32, kind="ExternalInput").ap()
    idx_d = dt("idx", [128, N_GRP * 128], i16, kind="ExternalInput").ap()
    numin_d = dt("numin", [128, BL], f32, kind="ExternalInput").ap()
    h0T_d = dt("h0T", [LH, BL], f32, kind="ExternalInput").ap()
    c0T_d = dt("c0T", [LH, BL], f32, kind="ExternalInput").ap()
    mask_d = dt("mask", [BL, 9], f32, kind="ExternalInput").ap()
    whhT_d = dt("whhT", [LH, NG * LH], f32, kind="ExternalInput").ap()
    numwT_d = dt("numwT", [128, ED], f32, kind="ExternalInput").ap()
    numb_d = dt("numb", [ED, 1], f32, kind="ExternalInput").ap()
    biasg_d = dt("biasg", [LH, NG], f32, kind="ExternalInput").ap()
    w1T_d = dt("w1T", [LH, 128], f32, kind="ExternalInput").ap()
    b1_d = dt("b1", [128, 1], f32, kind="ExternalInput").ap()
    w2T_d = dt("w2T", [128, ED], f32, kind="ExternalInput").ap()
    b2_d = dt("b2", [ED, 1], f32, kind="ExternalInput").ap()
    waT_d = dt("waT", [ED, 9], f32, kind="ExternalInput").ap()
    ba_d = dt("ba", [128, 9], f32, kind="ExternalInput").ap()
    out_d = dt("out_p", [BL, 9], f32, kind="ExternalOutput").ap()
    h1_d = dt("h1_p", [BL, LH], f32, kind="ExternalOutput").ap()
    c1_d = dt("c1_p", [BL, LH], f32, kind="ExternalOutput").ap()

    from contextlib import ExitStack

    with tile.TileContext(nc) as tc, ExitStack() as es:
        con = es.enter_context(tc.tile_pool(name="con", bufs=1))
        xgp = es.enter_context(tc.tile_pool(name="xgp", bufs=6))
        wihp = es.enter_context(tc.tile_pool(name="wihp", bufs=3))
        xtp = es.enter_context(tc.tile_pool(name="xtp", bufs=3))
        wrk = es.enter_context(tc.tile_pool(name="wrk", bufs=1))
        gps = es.enter_context(tc.tile_pool(name="gps", bufs=4, space="PSUM"))
        tps = es.enter_context(tc.tile_pool(name="tps", bufs=2, space="PSUM"))
        hps = es.enter_context(tc.tile_pool(name="hps", bufs=2, space="PSUM"))

        # ---- constants / small inputs ----
        ident = con.tile([128, 128], f32)
        make_identity(nc, ident)
        idx_sb = con.tile([128, N_GRP * 128], i16)
        nc.sync.dma_start(out=idx_sb, in_=idx_d)
        whhT = con.tile([LH, NG * LH], f32)
        nc.sync.dma_start(out=whhT, in_=whhT_d)
        h0T = con.tile([LH, BL], f32)
        nc.sync.dma_start(out=h0T, in_=h0T_d)
        c0T = con.tile([LH, BL], f32)
        nc.sync.dma_start(out=c0T, in_=c0T_d)
        numin = con.tile([128, BL], f32)
        nc.sync.dma_start(out=numin, in_=numin_d)
        numwT = con.tile([128, ED], f32)
        nc.sync.dma_start(out=numwT, in_=numwT_d)
        numb = con.tile([ED, 1], f32)
        nc.sync.dma_start(out=numb, in_=numb_d)
        biasg = con.tile([LH, NG], f32)
        nc.sync.dma_start(out=biasg, in_=biasg_d)
        w1T = con.tile([LH, 128], f32)
        nc.sync.dma_start(out=w1T, in_=w1T_d)
        b1 = con.tile([128, 1], f32)
        nc.sync.dma_start(out=b1, in_=b1_d)
        w2T = con.tile([128, ED], f32)
        nc.sync.dma_start(out=w2T, in_=w2T_d)
        b2 = con.tile([ED, 1], f32)
        nc.sync.dma_start(out=b2, in_=b2_d)
        waT = con.tile([ED, 9], f32)
        nc.sync.dma_start(out=waT, in_=waT_d)
        ba = con.tile([128, 9], f32)
        nc.sync.dma_start(out=ba, in_=ba_d)
        ones1_d = dt("ones1", [128, 128], f32, kind="ExternalInput").ap()
        ones1 = con.tile([128, 128], f32)
        nc.sync.dma_start(out=ones1, in_=ones1_d)
        mask_sb = con.tile([128, BCH, 9], f32)
        nc.sync.dma_start(
            out=mask_sb, in_=mask_d.rearrange("(g p) n -> p g n", p=128)
        )
        wih_num = con.tile([128, NG * LH], f32)
        nc.sync.dma_start(
            out=wih_num, in_=wihT[60, :, :]
        )

        # ---- gate accumulators (live across the whole k loop) ----
        gates = [
            gps.tile([128, BL], f32, tag="gates", name=f"gates{m}")
            for m in range(NG)
        ]

        # ---- numerical chunk: numT = num_W @ numerical^T + num_b ----
        numT_ps = tps.tile([128, BL], f32, tag="tp")
        nc.tensor.matmul(
            out=numT_ps, lhsT=_mm_ap(numwT), rhs=_mm_ap(numin),
            start=True, stop=True,
        )
        numT = wrk.tile([128, BL], f32, tag="numT")
        nc.scalar.activation(out=numT, in_=numT_ps, func=AF.Identity,
                             bias=numb[:, :])
        for m in range(NG):
            nc.tensor.matmul(
                out=gates[m],
                lhsT=_mm_ap(wih_num[:, m * LH:(m + 1) * LH]),
                rhs=_mm_ap(numT),
                start=True, stop=False,
            )
        # ---- recurrent term: Whh @ h0^T ----
        for m in range(NG):
            nc.tensor.matmul(
                out=gates[m],
                lhsT=_mm_ap(whhT[:, m * LH:(m + 1) * LH]),
                rhs=_mm_ap(h0T),
                start=False, stop=False,
            )

        # ---- main loop: gather 4 slots -> transpose -> 4x4 matmuls ----
        for gi in range(N_GRP):
            src = ab_emb if gi < 3 else mv_emb
            xg = xgp.tile([128, SLOT_G * BCH, 128], f32, tag="xg")
            nc.gpsimd.dma_gather(
                out_ap=xg[:, :, :],
                in_ap=src[:, :],
                idxs_ap=idx_sb[:, gi * 128:(gi + 1) * 128],
                num_idxs=IDX_PER_G,
                num_idxs_reg=IDX_PER_G,
                elem_size=ED,
                single_packet=False,
            )
            wih_g = wihp.tile([128, SLOT_G, NG * LH], f32, tag="wih")
            nc.sync.dma_start(
                out=wih_g,
                in_=wihT[gi * SLOT_G:(gi + 1) * SLOT_G, :, :].rearrange(
                    "k p n -> p k n"
                ),
            )
            for jj in range(SLOT_G):
                xt = xtp.tile([128, BL], f32, tag="xt")
                for g in range(BCH):
                    xt_ps = tps.tile([128, 128], f32, tag="tp")
                    nc.tensor.transpose(
                        out=xt_ps, in_=xg[:, jj * BCH + g, :], identity=ident
                    )
                    dst = xt[:, g * 128:(g + 1) * 128]
                    if (jj + g) % 2 == 0:
                        nc.vector.tensor_copy(out=dst, in_=xt_ps)
                    else:
                        nc.scalar.copy(out=dst, in_=xt_ps)
                last = gi == N_GRP - 1 and jj == SLOT_G - 1
                for m in range(NG):
                    nc.tensor.matmul(
                        out=gates[m],
                        lhsT=_mm_ap(wih_g[:, jj, m * LH:(m + 1) * LH]),
                        rhs=_mm_ap(xt),
                        start=False, stop=last,
                    )

        # ---- LSTM cell (feature-major [128, 512]) ----
        acts = []
        for m, fn in enumerate([AF.Sigmoid, AF.Sigmoid, AF.Tanh, AF.Sigmoid]):
            t = wrk.tile([128, BL], f32, tag=f"act{m}")
            nc.scalar.activation(out=t, in_=gates[m], func=fn,
                                 bias=biasg[:, m:m + 1])
            acts.append(t)
        i_s, f_s, g_s, o_s = acts
        ig = wrk.tile([128, BL], f32, tag="ig")
        nc.vector.tensor_mul(out=ig, in0=i_s, in1=g_s)
        fc = wrk.tile([128, BL], f32, tag="fc")
        nc.vector.tensor_mul(out=fc, in0=f_s, in1=c0T)
        c1T = wrk.tile([128, BL], f32, tag="c1T")
        nc.vector.tensor_add(out=c1T, in0=ig, in1=fc)
        tc1 = wrk.tile([128, BL], f32, tag="tc1")
        nc.scalar.activation(out=tc1, in_=c1T, func=AF.Tanh)
        h1T = wrk.tile([128, BL], f32, tag="h1T")
        nc.vector.tensor_mul(out=h1T, in0=o_s, in1=tc1)

        # ---- MLP head (feature-major) ----
        f1_ps = hps.tile([128, BL], f32, tag="hp")
        nc.tensor.matmul(out=f1_ps, lhsT=_mm_ap(w1T), rhs=_mm_ap(h1T),
                         start=True, stop=True)
        f1r = wrk.tile([128, BL], f32, tag="f1r")
        nc.scalar.activation(out=f1r, in_=f1_ps, func=AF.Relu, bias=b1[:, :])
        f2_ps = hps.tile([ED, BL], f32, tag="hp")
        nc.tensor.matmul(out=f2_ps, lhsT=_mm_ap(w2T), rhs=_mm_ap(f1r),
                         start=True, stop=True)
        f2T = wrk.tile([ED, BL], f32, tag="f2T")
        nc.scalar.activation(out=f2T, in_=f2_ps, func=AF.Identity,
                             bias=b2[:, :])

        # ---- logits + masked softmax, sample-major per 128-chunk ----
        out_sb = wrk.tile([128, BCH, 9], f32, tag="out_sb")
        for g in range(BCH):
            lg_ps = hps.tile([128, 9], f32, tag="hp")
            nc.tensor.matmul(out=lg_ps, lhsT=_mm_ap(ones1), rhs=_mm_ap(ba),
                             start=True, stop=False)
            nc.tensor.matmul(
                out=lg_ps, lhsT=_mm_ap(f2T[:, g * 128:(g + 1) * 128]),
                rhs=_mm_ap(waT), start=False, stop=True,
            )
            nmax = wrk.tile([128, 1], f32, tag=f"nmax{g}")
            nc.vector.reduce_max(out=nmax, in_=lg_ps,
                                 axis=mybir.AxisListType.X, negate=True)
            e = wrk.tile([128, 9], f32, tag=f"e{g}")
            nc.scalar.activation(out=e, in_=lg_ps, func=AF.Exp,
                                 bias=nmax[:, :])
            em = wrk.tile([128, 9], f32, tag=f"em{g}")
            nc.vector.tensor_mul(out=em, in0=e, in1=mask_sb[:, g, :])
            s_e = wrk.tile([128, 1], f32, tag=f"se{g}")
            nc.vector.reduce_sum(out=s_e, in_=e, axis=mybir.AxisListType.X)
            s_em = wrk.tile([128, 1], f32, tag=f"sem{g}")
            nc.vector.reduce_sum(out=s_em, in_=em, axis=mybir.AxisListType.X)
            z = wrk.tile([128, 1], f32, tag=f"z{g}")
            nc.vector.tensor_scalar(out=z, in0=s_em, scalar1=0.0, scalar2=None,
                                    op0=mybir.AluOpType.is_le)
            ze = wrk.tile([128, 9], f32, tag=f"ze{g}")
            nc.vector.tensor_scalar_mul(out=ze, in0=e, scalar1=z[:, :])
            numer = wrk.tile([128, 9], f32, tag=f"num{g}")
            nc.vector.tensor_add(out=numer, in0=em, in1=ze)
            zs = wrk.tile([128, 1], f32, tag=f"zs{g}")
            nc.vector.tensor_mul(out=zs, in0=z, in1=s_e)
            den = wrk.tile([128, 1], f32, tag=f"den{g}")
            nc.vector.tensor_add(out=den, in0=s_em, in1=zs)
            rden = wrk.tile([128, 1], f32, tag=f"rden{g}")
            nc.vector.reciprocal(out=rden, in_=den)
            nc.vector.tensor_scalar_mul(out=out_sb[:, g, :], in0=numer,
                                        scalar1=rden[:, :])
        nc.sync.dma_start(
            out=out_d.rearrange("(g p) n -> p g n", p=128), in_=out_sb
        )

        # ---- h1/c1 back to sample-major ----
        for name, src_t, dst_d in (("h1", h1T, h1_d), ("c1", c1T, c1_d)):
            sm = wrk.tile([128, BCH, LH], f32, tag=f"{name}sm")
            for g in range(BCH):
                tr_ps = hps.tile([128, 128], f32, tag="hp")
                nc.tensor.transpose(
                    out=tr_ps, in_=src_t[:, g * 128:(g + 1) * 128],
                    identity=ident,
                )
                if g % 2 == 0:
                    nc.vector.tensor_copy(out=sm[:, g, :], in_=tr_ps)
                else:
                    nc.scalar.copy(out=sm[:, g, :], in_=tr_ps)
            nc.sync.dma_start(
                out=dst_d.rearrange("(g p) n -> p g n", p=128), in_=sm
            )

    nc.compile()
    _CACHE[key] = nc
    return nc


def _wrap_idx(ids):
    """ids: [SLOT_G, BL] int array (slot-major) -> wrapped [128, 128] int16."""
    flat = ids.reshape(-1).astype(np.int16)          # n = jj*BL + b
    w = flat.reshape(IDX_PER_G // 16, 16).T          # [16, 128]
    return np.tile(w, (8, 1))                        # replicate to 128 parts


def _padp(a, p):
    """zero-pad partition (first) dim to p rows"""
    out = np.zeros((p,) + a.shape[1:], a.dtype)
    out[: a.shape[0]] = a
    return out


def _padf(a, n):
    """zero-pad second dim to n cols"""
    out = np.zeros((a.shape[0], n) + a.shape[2:], a.dtype)
    out[:, : a.shape[1]] = a
    return out


def _ones_row0():
    o = np.zeros((128, 128), np.float32)
    o[0, :] = 1.0
    return o


def _prep(inputs):
    """Host-side formatting: shard + transpose weights into kernel layouts."""
    ability_ids = np.asarray(inputs["ability_ids"])
    move_ids = np.asarray(inputs["move_ids"])
    numerical = np.asarray(inputs["numerical"], dtype=np.float32)
    mask = np.asarray(inputs["mask"], dtype=np.float32)
    h0 = np.asarray(inputs["h0"], dtype=np.float32)
    c0 = np.asarray(inputs["c0"], dtype=np.float32)
    Wih = np.asarray(inputs["Wih"], dtype=np.float32)
    Whh = np.asarray(inputs["Whh"], dtype=np.float32)

    shared = {
        "wihT": np.ascontiguousarray(Wih.T).reshape(K_CHUNKS, 128, NG * LH),
        "ab_emb": np.ascontiguousarray(inputs["ability_emb"], dtype=np.float32),
        "mv_emb": np.ascontiguousarray(inputs["move_emb"], dtype=np.float32),
        "whhT": np.ascontiguousarray(Whh.T),
        "numwT": _padp(np.ascontiguousarray(
            np.asarray(inputs["num_W"], dtype=np.float32).T), 128),
        "numb": np.asarray(inputs["num_b"], dtype=np.float32).reshape(ED, 1),
        "biasg": np.ascontiguousarray(
            (np.asarray(inputs["bih"], dtype=np.float32)
             + np.asarray(inputs["bhh"], dtype=np.float32)
             ).reshape(NG, LH).T),
        "w1T": _padf(np.ascontiguousarray(
            np.asarray(inputs["W1"], dtype=np.float32).T), 128),
        "b1": _padp(
            np.asarray(inputs["b1"], dtype=np.float32).reshape(HD, 1), 128),
        "w2T": _padp(np.ascontiguousarray(
            np.asarray(inputs["W2"], dtype=np.float32).T), 128),
        "b2": np.asarray(inputs["b2"], dtype=np.float32).reshape(ED, 1),
        "waT": np.ascontiguousarray(
            np.asarray(inputs["Wa"], dtype=np.float32).T),
        "ba": _padp(
            np.asarray(inputs["ba"], dtype=np.float32).reshape(1, 9), 128),
        "ones1": _ones_row0(),
    }

    in_maps = []
    for c in range(N_CORES):
        s = slice(c * BL, (c + 1) * BL)
        idx_np = np.empty((128, N_GRP * 128), np.int16)
        for gi in range(N_GRP):
            if gi < 3:
                cols = ability_ids[s, gi * SLOT_G:(gi + 1) * SLOT_G]
            else:
                j0 = (gi - 3) * SLOT_G
                cols = move_ids[s, j0:j0 + SLOT_G]
            idx_np[:, gi * 128:(gi + 1) * 128] = _wrap_idx(
                np.ascontiguousarray(cols.T))
        m = dict(shared)
        m["idx"] = idx_np
        m["numin"] = _padp(np.ascontiguousarray(numerical[s].T), 128)
        m["h0T"] = np.ascontiguousarray(h0[s].T)
        m["c0T"] = np.ascontiguousarray(c0[s].T)
        m["mask"] = np.ascontiguousarray(mask[s])
        in_maps.append(m)
    return in_maps


def run_sharded(inputs, **kw):
    nc = _build()
    in_maps = _prep(inputs)
    res = bass_utils.run_bass_kernel_spmd(
        nc, in_maps, core_ids=list(range(N_CORES)), **kw
    )
    out = np.concatenate([r["out_p"] for r in res.results], axis=0)
    h1 = np.concatenate([r["h1_p"] for r in res.results], axis=0)
    c1 = np.concatenate([r["c1_p"] for r in res.results], axis=0)
    return (out, h1, c1), res


def kernel(**inputs):
    outs, _ = run_sharded(inputs)
    return outs


# revision 8
# speedup vs baseline: 1.3019x; 1.3019x over previous
"""Trainium2 Bass kernel for nn_CustomPokemonModel.

Model (per sample): embedding gathers (12 ability + 48 move slots) + small
linear on numericals -> concat x [B, 7808] -> single-step LSTM (4*128 gates)
-> 2-layer MLP head -> 9-way masked softmax.  Returns (out, h1, c1).

Strategy: pure data parallel over 8 NeuronCores (512 samples each).  On each
core the whole pipeline is computed feature-major ("xT" layout: features on
partitions, samples on the free dim) so every matmul contracts over the
partition dim with B=512 on the moving free dim:

  - embeddings are gathered row-wise with dma_gather (sample-major blocks of
    [128 samples, 128 feat]) and transposed on the PE with an identity matmul
  - gates^T [4x128, 512] accumulate in 4 PSUM banks over 61 k-chunks of
    Wih^T (streamed from DRAM) plus the Whh @ h0^T term
  - LSTM elementwise + MLP head stay feature-major; the final 9-way logits
    flip back to sample-major by using feat2^T 128-sample chunks as the
    stationary operand; softmax + mask renorm run on DVE/ACT
  - h1/c1 are PE-transposed back to sample-major for output

MM_DTYPE selects the PE dtype for the big matmuls: float32 (exact, 4 cyc/row)
or float32r (full-speed fp32-storage mode, 1 cyc/row at N>=512).
"""
import sys

sys.path.insert(0, "/opt/trn_rl_repo")

import numpy as np

import concourse.bass as bass
import concourse.tile as tile
from concourse import bacc, bass_utils, mybir

f32 = mybir.dt.float32
i16 = mybir.dt.int16
AF = mybir.ActivationFunctionType

N_CORES = 8
B = 4096
BL = B // N_CORES          # 512 samples per core
ED = 128
LH = 128
HD = 64
NG = 4                      # gate chunks (i, f, g, o)
N_AB, N_MV = 12, 48
K_CHUNKS = 61               # 12 ability + 48 move + 1 numerical
N_GRP = 15                  # gather/matmul groups of 4 slots
SLOT_G = 4                  # slots per group
IDX_PER_G = SLOT_G * BL     # 2048 indices per dma_gather
BCH = BL // 128             # 4 sample chunks of 128

# "f32" = exact fp32 matmuls; "f32r" = float32r (fast fp32-storage) matmuls
MM_DTYPE = "f32r"


def _xdt():
    """dtype of the gathered-x / Wih matmul path"""
    return mybir.dt.float32r if MM_DTYPE == "f32r" else f32


def _round_f32r(a):
    """round-to-nearest-even fp32 -> fp32r (8-bit exp, 11-bit mantissa,
    value kept in the top 20 bits of the fp32 word)"""
    if MM_DTYPE != "f32r":
        return a
    u = np.ascontiguousarray(a, dtype=np.float32).view(np.uint32)
    u = (u + 0x7FF + ((u >> 12) & 1)) & np.uint32(0xFFFFF000)
    return u.view(np.float32)


_CACHE = {}


def _build():
    key = MM_DTYPE
    if key in _CACHE:
        return _CACHE[key]

    nc = bacc.Bacc("TRN2", target_bir_lowering=False, debug=False,
                   num_swdge_queues=4)

    dt = nc.dram_tensor
    xdt = _xdt()
    wihT = dt("wihT", [60, 128, NG * LH], xdt, kind="ExternalInput").ap()
    wihnum_d = dt("wihnum", [128, NG * LH], f32, kind="ExternalInput").ap()
    identx_d = dt("identx", [128, 128], xdt, kind="ExternalInput").ap()
    idento_d = dt("idento", [128, 128], f32, kind="ExternalInput").ap()
    ab_emb = dt("ab_emb", [300, ED], xdt, kind="ExternalInput").ap()
    mv_emb = dt("mv_emb", [900, ED], xdt, kind="ExternalInput").ap()
    idx_d = dt("idx", [128, N_GRP * 128], i16, kind="ExternalInput").ap()
    numin_d = dt("numin", [128, BL], f32, kind="ExternalInput").ap()
    h0T_d = dt("h0T", [LH, BL], f32, kind="ExternalInput").ap()
    c0T_d = dt("c0T", [LH, BL], f32, kind="ExternalInput").ap()
    mask_d = dt("mask", [BL, 9], f32, kind="ExternalInput").ap()
    whhT_d = dt("whhT", [LH, NG * LH], f32, kind="ExternalInput").ap()
    numwT_d = dt("numwT", [128, ED], f32, kind="ExternalInput").ap()
    numb_d = dt("numb", [ED, 1], f32, kind="ExternalInput").ap()
    biasg_d = dt("biasg", [LH, NG], f32, kind="ExternalInput").ap()
    w1T_d = dt("w1T", [LH, 128], f32, kind="ExternalInput").ap()
    b1_d = dt("b1", [128, 1], f32, kind="ExternalInput").ap()
    w2T_d = dt("w2T", [128, ED], f32, kind="ExternalInput").ap()
    b2_d = dt("b2", [ED, 1], f32, kind="ExternalInput").ap()
    waT_d = dt("waT", [ED, 9], f32, kind="ExternalInput").ap()
    ba_d = dt("ba", [128, 9], f32, kind="ExternalInput").ap()
    out_d = dt("out_p", [BL, 9], f32, kind="ExternalOutput").ap()
    h1_d = dt("h1_p", [BL, LH], f32, kind="ExternalOutput").ap()
    c1_d = dt("c1_p", [BL, LH], f32, kind="ExternalOutput").ap()

    from contextlib import ExitStack

    with tile.TileContext(nc) as tc, ExitStack() as es:
        con = es.enter_context(tc.tile_pool(name="con", bufs=1))
        xgp = es.enter_context(tc.tile_pool(name="xgp", bufs=6))
        wihp = es.enter_context(tc.tile_pool(name="wihp", bufs=3))
        xtp = es.enter_context(tc.tile_pool(name="xtp", bufs=3))
        wrk = es.enter_context(tc.tile_pool(name="wrk", bufs=1))
        gps = es.enter_context(tc.tile_pool(name="gps", bufs=4, space="PSUM"))
        tps = es.enter_context(tc.tile_pool(name="tps", bufs=2, space="PSUM"))
        hps = es.enter_context(tc.tile_pool(name="hps", bufs=2, space="PSUM"))

        # ---- constants / small inputs ----
        identx = con.tile([128, 128], xdt)
        nc.sync.dma_start(out=identx, in_=identx_d)
        idento = con.tile([128, 128], f32)
        nc.sync.dma_start(out=idento, in_=idento_d)
        idx_sb = con.tile([128, N_GRP * 128], i16)
        nc.sync.dma_start(out=idx_sb, in_=idx_d)
        whhT = con.tile([LH, NG * LH], f32)
        nc.sync.dma_start(out=whhT, in_=whhT_d)
        h0T = con.tile([LH, BL], f32)
        nc.sync.dma_start(out=h0T, in_=h0T_d)
        c0T = con.tile([LH, BL], f32)
        nc.sync.dma_start(out=c0T, in_=c0T_d)
        numin = con.tile([128, BL], f32)
        nc.sync.dma_start(out=numin, in_=numin_d)
        numwT = con.tile([128, ED], f32)
        nc.sync.dma_start(out=numwT, in_=numwT_d)
        numb = con.tile([ED, 1], f32)
        nc.sync.dma_start(out=numb, in_=numb_d)
        biasg = con.tile([LH, NG], f32)
        nc.sync.dma_start(out=biasg, in_=biasg_d)
        w1T = con.tile([LH, 128], f32)
        nc.sync.dma_start(out=w1T, in_=w1T_d)
        b1 = con.tile([128, 1], f32)
        nc.sync.dma_start(out=b1, in_=b1_d)
        w2T = con.tile([128, ED], f32)
        nc.sync.dma_start(out=w2T, in_=w2T_d)
        b2 = con.tile([ED, 1], f32)
        nc.sync.dma_start(out=b2, in_=b2_d)
        waT = con.tile([ED, 9], f32)
        nc.sync.dma_start(out=waT, in_=waT_d)
        ba = con.tile([128, 9], f32)
        nc.sync.dma_start(out=ba, in_=ba_d)
        ones1_d = dt("ones1", [128, 128], f32, kind="ExternalInput").ap()
        ones1 = con.tile([128, 128], f32)
        nc.sync.dma_start(out=ones1, in_=ones1_d)
        mask_sb = con.tile([128, BCH, 9], f32)
        nc.sync.dma_start(
            out=mask_sb, in_=mask_d.rearrange("(g p) n -> p g n", p=128)
        )
        wih_num = con.tile([128, NG * LH], f32)
        nc.sync.dma_start(out=wih_num, in_=wihnum_d)

        # ---- gate accumulators (live across the whole k loop) ----
        gates = [
            gps.tile([128, BL], f32, tag="gates", name=f"gates{m}")
            for m in range(NG)
        ]

        # ---- numerical chunk: numT = num_W @ numerical^T + num_b ----
        numT_ps = tps.tile([128, BL], f32, tag="tp")
        nc.tensor.matmul(
            out=numT_ps, lhsT=numwT, rhs=numin,
            start=True, stop=True,
        )
        numT = wrk.tile([128, BL], f32, tag="numT")
        nc.scalar.activation(out=numT, in_=numT_ps, func=AF.Identity,
                             bias=numb[:, :])
        for m in range(NG):
            nc.tensor.matmul(
                out=gates[m],
                lhsT=wih_num[:, m * LH:(m + 1) * LH],
                rhs=numT,
                start=True, stop=False,
            )
        # ---- recurrent term: Whh @ h0^T ----
        for m in range(NG):
            nc.tensor.matmul(
                out=gates[m],
                lhsT=whhT[:, m * LH:(m + 1) * LH],
                rhs=h0T,
                start=False, stop=False,
            )

        # ---- main loop: gather 4 slots -> transpose -> 4x4 matmuls ----
        for gi in range(N_GRP):
            src = ab_emb if gi < 3 else mv_emb
            xg = xgp.tile([128, SLOT_G * BCH, 128], xdt, tag="xg")
            nc.gpsimd.dma_gather(
                out_ap=xg[:, :, :],
                in_ap=src[:, :],
                idxs_ap=idx_sb[:, gi * 128:(gi + 1) * 128],
                num_idxs=IDX_PER_G,
                num_idxs_reg=IDX_PER_G,
                elem_size=ED,
                single_packet=False,
                queue_num=gi % 4,
            )
            wih_g = wihp.tile([128, SLOT_G, NG * LH], xdt, tag="wih")
            nc.sync.dma_start(
                out=wih_g,
                in_=wihT[gi * SLOT_G:(gi + 1) * SLOT_G, :, :].rearrange(
                    "k p n -> p k n"
                ),
            )
            for jj in range(SLOT_G):
                xt = xtp.tile([128, BL], xdt, tag="xt")
                for g in range(BCH):
                    xt_ps = tps.tile([128, 128], xdt, tag="tp")
                    nc.tensor.transpose(
                        out=xt_ps, in_=xg[:, jj * BCH + g, :], identity=identx
                    )
                    dst = xt[:, g * 128:(g + 1) * 128]
                    if (jj + g) % 2 == 0:
                        nc.vector.tensor_copy(out=dst, in_=xt_ps)
                    else:
                        nc.scalar.copy(out=dst, in_=xt_ps)
                last = gi == N_GRP - 1 and jj == SLOT_G - 1
                for m in range(NG):
                    nc.tensor.matmul(
                        out=gates[m],
                        lhsT=wih_g[:, jj, m * LH:(m + 1) * LH],
                        rhs=xt,
                        start=False, stop=last,
                    )

        # ---- LSTM cell (feature-major [128, 512]) ----
        acts = []
        for m, fn in enumerate([AF.Sigmoid, AF.Sigmoid, AF.Tanh, AF.Sigmoid]):
            t = wrk.tile([128, BL], f32, tag=f"act{m}")
            nc.scalar.activation(out=t, in_=gates[m], func=fn,
                                 bias=biasg[:, m:m + 1])
            acts.append(t)
        i_s, f_s, g_s, o_s = acts
        ig = wrk.tile([128, BL], f32, tag="ig")
        nc.vector.tensor_mul(out=ig, in0=i_s, in1=g_s)
        fc = wrk.tile([128, BL], f32, tag="fc")
        nc.vector.tensor_mul(out=fc, in0=f_s, in1=c0T)
        c1T = wrk.tile([128, BL], f32, tag="c1T")
        nc.vector.tensor_add(out=c1T, in0=ig, in1=fc)
        tc1 = wrk.tile([128, BL], f32, tag="tc1")
        nc.scalar.activation(out=tc1, in_=c1T, func=AF.Tanh)
        h1T = wrk.tile([128, BL], f32, tag="h1T")
        nc.vector.tensor_mul(out=h1T, in0=o_s, in1=tc1)

        # ---- MLP head (feature-major) ----
        f1_ps = hps.tile([128, BL], f32, tag="hp")
        nc.tensor.matmul(out=f1_ps, lhsT=w1T, rhs=h1T,
                         start=True, stop=True)
        f1r = wrk.tile([128, BL], f32, tag="f1r")
        nc.scalar.activation(out=f1r, in_=f1_ps, func=AF.Relu, bias=b1[:, :])
        f2_ps = hps.tile([ED, BL], f32, tag="hp")
        nc.tensor.matmul(out=f2_ps, lhsT=w2T, rhs=f1r,
                         start=True, stop=True)
        f2T = wrk.tile([ED, BL], f32, tag="f2T")
        nc.scalar.activation(out=f2T, in_=f2_ps, func=AF.Identity,
                             bias=b2[:, :])

        # ---- logits + masked softmax, sample-major per 128-chunk ----
        out_sb = wrk.tile([128, BCH, 9], f32, tag="out_sb")
        for g in range(BCH):
            lg_ps = hps.tile([128, 9], f32, tag="hp")
            nc.tensor.matmul(out=lg_ps, lhsT=ones1, rhs=ba,
                             start=True, stop=False)
            nc.tensor.matmul(
                out=lg_ps, lhsT=f2T[:, g * 128:(g + 1) * 128],
                rhs=waT, start=False, stop=True,
            )
            nmax = wrk.tile([128, 1], f32, tag=f"nmax{g}")
            nc.vector.reduce_max(out=nmax, in_=lg_ps,
                                 axis=mybir.AxisListType.X, negate=True)
            e = wrk.tile([128, 9], f32, tag=f"e{g}")
            nc.scalar.activation(out=e, in_=lg_ps, func=AF.Exp,
                                 bias=nmax[:, :])
            em = wrk.tile([128, 9], f32, tag=f"em{g}")
            nc.vector.tensor_mul(out=em, in0=e, in1=mask_sb[:, g, :])
            s_e = wrk.tile([128, 1], f32, tag=f"se{g}")
            nc.vector.reduce_sum(out=s_e, in_=e, axis=mybir.AxisListType.X)
            s_em = wrk.tile([128, 1], f32, tag=f"sem{g}")
            nc.vector.reduce_sum(out=s_em, in_=em, axis=mybir.AxisListType.X)
            z = wrk.tile([128, 1], f32, tag=f"z{g}")
            nc.vector.tensor_scalar(out=z, in0=s_em, scalar1=0.0, scalar2=None,
                                    op0=mybir.AluOpType.is_le)
            ze = wrk.tile([128, 9], f32, tag=f"ze{g}")
            nc.vector.tensor_scalar_mul(out=ze, in0=e, scalar1=z[:, :])
            numer = wrk.tile([128, 9], f32, tag=f"num{g}")
            nc.vector.tensor_add(out=numer, in0=em, in1=ze)
            zs = wrk.tile([128, 1], f32, tag=f"zs{g}")
            nc.vector.tensor_mul(out=zs, in0=z, in1=s_e)
            den = wrk.tile([128, 1], f32, tag=f"den{g}")
            nc.vector.tensor_add(out=den, in0=s_em, in1=zs)
            rden = wrk.tile([128, 1], f32, tag=f"rden{g}")
            nc.vector.reciprocal(out=rden, in_=den)
            nc.vector.tensor_scalar_mul(out=out_sb[:, g, :], in0=numer,
                                        scalar1=rden[:, :])
        nc.sync.dma_start(
            out=out_d.rearrange("(g p) n -> p g n", p=128), in_=out_sb
        )

        # ---- h1/c1 back to sample-major ----
        for name, src_t, dst_d in (("h1", h1T, h1_d), ("c1", c1T, c1_d)):
            sm = wrk.tile([128, BCH, LH], f32, tag=f"{name}sm")
            for g in range(BCH):
                tr_ps = hps.tile([128, 128], f32, tag="hp")
                nc.tensor.transpose(
                    out=tr_ps, in_=src_t[:, g * 128:(g + 1) * 128],
                    identity=idento,
                )
                if g % 2 == 0:
                    nc.vector.tensor_copy(out=sm[:, g, :], in_=tr_ps)
                else:
                    nc.scalar.copy(out=sm[:, g, :], in_=tr_ps)
            nc.sync.dma_start(
                out=dst_d.rearrange("(g p) n -> p g n", p=128), in_=sm
            )

    nc.compile()
    _CACHE[key] = nc
    return nc


def _wrap_idx(ids):
    """ids: [SLOT_G, BL] int array (slot-major) -> wrapped [128, 128] int16."""
    flat = ids.reshape(-1).astype(np.int16)          # n = jj*BL + b
    w = flat.reshape(IDX_PER_G // 16, 16).T          # [16, 128]
    return np.tile(w, (8, 1))                        # replicate to 128 parts


def _padp(a, p):
    """zero-pad partition (first) dim to p rows"""
    out = np.zeros((p,) + a.shape[1:], a.dtype)
    out[: a.shape[0]] = a
    return out


def _padf(a, n):
    """zero-pad second dim to n cols"""
    out = np.zeros((a.shape[0], n) + a.shape[2:], a.dtype)
    out[:, : a.shape[1]] = a
    return out


def _ones_row0():
    o = np.zeros((128, 128), np.float32)
    o[0, :] = 1.0
    return o


def _prep(inputs):
    """Host-side formatting: shard + transpose weights into kernel layouts."""
    ability_ids = np.asarray(inputs["ability_ids"])
    move_ids = np.asarray(inputs["move_ids"])
    numerical = np.asarray(inputs["numerical"], dtype=np.float32)
    mask = np.asarray(inputs["mask"], dtype=np.float32)
    h0 = np.asarray(inputs["h0"], dtype=np.float32)
    c0 = np.asarray(inputs["c0"], dtype=np.float32)
    Wih = np.asarray(inputs["Wih"], dtype=np.float32)
    Whh = np.asarray(inputs["Whh"], dtype=np.float32)

    shared = {
        "wihT": _round_f32r(
            np.ascontiguousarray(Wih.T).reshape(K_CHUNKS, 128, NG * LH)[:60]),
        "wihnum": np.ascontiguousarray(Wih.T[60 * 128:]),
        "identx": np.eye(128, dtype=np.float32),
        "idento": np.eye(128, dtype=np.float32),
        "ab_emb": _round_f32r(
            np.ascontiguousarray(inputs["ability_emb"], dtype=np.float32)),
        "mv_emb": _round_f32r(
            np.ascontiguousarray(inputs["move_emb"], dtype=np.float32)),
        "whhT": np.ascontiguousarray(Whh.T),
        "numwT": _padp(np.ascontiguousarray(
            np.asarray(inputs["num_W"], dtype=np.float32).T), 128),
        "numb": np.asarray(inputs["num_b"], dtype=np.float32).reshape(ED, 1),
        "biasg": np.ascontiguousarray(
            (np.asarray(inputs["bih"], dtype=np.float32)
             + np.asarray(inputs["bhh"], dtype=np.float32)
             ).reshape(NG, LH).T),
        "w1T": _padf(np.ascontiguousarray(
            np.asarray(inputs["W1"], dtype=np.float32).T), 128),
        "b1": _padp(
            np.asarray(inputs["b1"], dtype=np.float32).reshape(HD, 1), 128),
        "w2T": _padp(np.ascontiguousarray(
            np.asarray(inputs["W2"], dtype=np.float32).T), 128),
        "b2": np.asarray(inputs["b2"], dtype=np.float32).reshape(ED, 1),
        "waT": np.ascontiguousarray(
            np.asarray(inputs["Wa"], dtype=np.float32).T),
        "ba": _padp(
            np.asarray(inputs["ba"], dtype=np.float32).reshape(1, 9), 128),
        "ones1": _ones_row0(),
    }

    in_maps = []
    for c in range(N_CORES):
        s = slice(c * BL, (c + 1) * BL)
        idx_np = np.empty((128, N_GRP * 128), np.int16)
        for gi in range(N_GRP):
            if gi < 3:
                cols = ability_ids[s, gi * SLOT_G:(gi + 1) * SLOT_G]
            else:
                j0 = (gi - 3) * SLOT_G
                cols = move_ids[s, j0:j0 + SLOT_G]
            idx_np[:, gi * 128:(gi + 1) * 128] = _wrap_idx(
                np.ascontiguousarray(cols.T))
        m = dict(shared)
        m["idx"] = idx_np
        m["numin"] = _padp(np.ascontiguousarray(numerical[s].T), 128)
        m["h0T"] = np.ascontiguousarray(h0[s].T)
        m["c0T"] = np.ascontiguousarray(c0[s].T)
        m["mask"] = np.ascontiguousarray(mask[s])
        in_maps.append(m)
    return in_maps


def run_sharded(inputs, **kw):
    nc = _build()
    in_maps = _prep(inputs)
    res = bass_utils.run_bass_kernel_spmd(
        nc, in_maps, core_ids=list(range(N_CORES)), **kw
    )
    out = np.concatenate([r["out_p"] for r in res.results], axis=0)
    h1 = np.concatenate([r["h1_p"] for r in res.results], axis=0)
    c1 = np.concatenate([r["c1_p"] for r in res.results], axis=0)
    return (out, h1, c1), res


def kernel(**inputs):
    outs, _ = run_sharded(inputs)
    return outs
